# revision 38
# baseline (speedup 1.0000x reference)
"""Distributed Trainium2 kernel for the LN->silu->QKV(+LN on q,k)->attention->silu->proj block.

Sharding: sequence-parallel over 8 NeuronCores. Each core owns 512 of the 4096
tokens; both LayerNorms are per-token so they stay local. Collectives are four
per-head-pair AllGathers of (K^T, V_aug) so attention on pair p can start as
soon as its own gather lands.

Device layout conventions (per core):
  natural    = [token partitions, feature free]
  transposed = [feature partitions, token free]
Scores are computed transposed (S^T = [key, query]) so the softmax denominators
come free from the P@V matmul: V is augmented with a ones column, so the AV
accumulator row 64 is sum_k P. exp() needs no max subtraction: q,k are
LayerNorm outputs and q is scaled by inner^-0.5, so |scores| <~ 3.

Engine split: all matmuls bf16 on PE; LayerNorm rsqrt via a DVE bit-trick
(fast inverse sqrt + 1 Newton step) so ACT only ever needs the exp_and_others
table set (exp + tanh); softmax exp runs on ACT for 5 heads and on DVE
(Schraudolph bit-trick exp, ~3% max err) for 3 heads to balance the two
engines. silu is z*(1+tanh(z/2)) = 2*silu(z) with the 0.5 folded into the
host-scaled weights.

Attention inner loop is software-pipelined (scores(b); exp(b); AV(b-1)) so the
in-order PE queue never stalls waiting for exp.
"""

import sys
import numpy as np

sys.path.insert(0, "/opt/trn_rl_repo")

import concourse.bacc as bacc  # noqa: E402
import concourse.tile as tile  # noqa: E402
from concourse import mybir, masks  # noqa: E402
from concourse.bass_utils import run_bass_kernel_spmd  # noqa: E402

FP = mybir.dt.float32
BF = mybir.dt.bfloat16
F8 = mybir.dt.float8e4
I32 = mybir.dt.int32
AF = mybir.ActivationFunctionType
ALU = mybir.AluOpType

NC = 8          # cores
P = 128         # partitions
N = 4096        # sequence
C = 512         # channels
INNER = 512     # heads * dim_head
H = 8           # heads
D = 64          # dim per head
TLOC = N // NC  # tokens per core (512)
NJ = TLOC // P  # token tiles per core (4)
NCH = C // P    # channel chunks (4)
VW = D + 1      # augmented v width (65)
BLK = 3         # key chunks per score block (3 PSUM banks per head)
CHUNKS = N // P  # 32 key chunks

# (pair, hh) whose softmax exp runs on DVE via the Schraudolph bit trick.
EXP_DVE = {(0, 1), (1, 1), (2, 1), (3, 1)}
SCH_A = float(1 << 23) / float(np.log(2.0))
SCH_B = float(127 * (1 << 23) - 366393)

_CACHE = {}


def _fast_rsqrt(nc, pool, var_ap, tag):
    """rs = 1/sqrt(var) on DVE: magic-constant seed + one Newton step.
    var_ap: [P, NJ] fp32 SBUF. Returns [P, NJ] fp32 tile. Max rel err ~0.2%."""
    shp = list(var_ap.shape)
    ish = pool.tile(shp, I32, tag=f"{tag}_i", name=f"{tag}_i")
    nc.vector.tensor_scalar(
        ish[:], var_ap.bitcast(I32), 1, None, ALU.logical_shift_right
    )
    r0 = pool.tile(shp, I32, tag=f"{tag}_r0", name=f"{tag}_r0")
    nc.vector.tensor_scalar(r0[:], ish[:], -1, 0x5F3759DF, ALU.mult, ALU.add)
    vr = pool.tile(shp, FP, tag=f"{tag}_vr", name=f"{tag}_vr")
    nc.vector.tensor_tensor(vr[:], var_ap, r0[:].bitcast(FP), ALU.mult)
    vrr = pool.tile(shp, FP, tag=f"{tag}_v2", name=f"{tag}_v2")
    nc.vector.tensor_tensor(vrr[:], vr[:], r0[:].bitcast(FP), ALU.mult)
    h = pool.tile(shp, FP, tag=f"{tag}_h", name=f"{tag}_h")
    nc.vector.tensor_scalar(h[:], vrr[:], -0.5, 1.5, ALU.mult, ALU.add)
    rs = pool.tile(shp, FP, tag=f"{tag}_rs", name=f"{tag}_rs")
    nc.vector.tensor_tensor(rs[:], h[:], r0[:].bitcast(FP), ALU.mult)
    return rs


def _ln_coeffs(nc, pool, src_ap, tag):
    """Per-token LN coefficients for src_ap [128, NJ, 512]: returns
    (rs, nmr) [128, NJ]: rsqrt(var) and -mean*rsqrt (eps skipped; var >> eps)."""
    stats = pool.tile([P, NJ, 6], FP, tag=f"{tag}_st", name=f"{tag}_st")
    for j in range(NJ):
        nc.vector.bn_stats(stats[:, j, :], src_ap[:, j, :])
    aggr = pool.tile([P, NJ, 2], FP, tag=f"{tag}_ag", name=f"{tag}_ag")
    for j in range(NJ):
        nc.vector.bn_aggr(aggr[:, j, :], stats[:, j, :])
    rs = _fast_rsqrt(nc, pool, aggr[:, :, 1], f"{tag}_q")
    nmr = pool.tile([P, NJ], FP, tag=f"{tag}_nm", name=f"{tag}_nm")
    nc.vector.scalar_tensor_tensor(
        nmr[:], aggr[:, :, 0], -1.0, rs[:], ALU.mult, ALU.mult
    )
    return rs, nmr


def build_graph():
    nc = bacc.Bacc("TRN2", target_bir_lowering=False, debug=False, num_devices=NC)

    x_in = nc.dram_tensor("x", [TLOC, C], FP, kind="ExternalInput")
    w_in = {}
    for nm in ("wq", "wk", "wv", "wo"):
        w_in[nm] = nc.dram_tensor(nm, [C, C], BF, kind="ExternalInput")
    row_in = {}
    for nm in ("bq", "bk", "bv", "bo"):
        row_in[nm] = nc.dram_tensor(nm, [1, C], BF, kind="ExternalInput")
    for nm in ("gq", "beq", "gk", "bek"):
        row_in[nm] = nc.dram_tensor(nm, [1, C], FP, kind="ExternalInput")
    out_ext = nc.dram_tensor("out", [TLOC, C], FP, kind="ExternalOutput")

    # per-pair collective bounce/gather buffers. fp8 halves the wire bytes
    # (the AllGather is the latency+bandwidth wall) and its 1-byte elements
    # let k^T [128,512] and v_aug [512,130] pack evenly into one 512B-wide
    # buffer, so each pair needs only ONE collective (control overhead is
    # ~8us per collective).
    KROWS = P            # k^T block rows in kvb
    VROWS = (TLOC * 2 * VW) // C  # 130 rows of flattened v bytes
    KVR = KROWS + VROWS  # 258
    kvb = [nc.dram_tensor(f"kvb{p}", [KVR, C], F8) for p in range(H // 2)]
    kvg = [nc.dram_tensor(f"kvg{p}", [NC * KVR, C], F8, addr_space="Shared")
           for p in range(H // 2)]

    with tile.TileContext(nc) as tc:
        with tc.tile_pool(name="persist", bufs=1) as pers:
            ident = pers.tile([P, P], BF)
            masks.make_identity(nc, ident[:])
            ones_r = pers.tile([1, P], BF)
            nc.vector.memset(ones_r[:], 1.0)

            # qT[c]: [inner partitions, token free]; chunk c = head pair c
            qT = [pers.tile([P, NJ, P], BF, tag=f"qT{c}", name=f"qT{c}")
                  for c in range(NCH)]
            soT = [pers.tile([P, NJ, P], BF, tag=f"soT{c}", name=f"soT{c}")
                   for c in range(NCH)]
            kT_loc = pers.tile([P, NCH, NJ, P], F8)   # k^T local by chunk
            vaug = pers.tile([P, NJ, H, VW], F8)      # v augmented, natural

            # ---------------- phase 1: local projections ----------------
            with tc.tile_pool(name="ph1", bufs=1) as ph1, \
                 tc.tile_pool(name="ph1ps", bufs=1, space="PSUM") as ph1ps, \
                 tc.tile_pool(name="ph1pv", bufs=2, space="PSUM") as ph1pv, \
                 tc.tile_pool(name="ph1tr", bufs=2, space="PSUM") as ph1tr, \
                 tc.tile_pool(name="ph1sm", bufs=2) as ph1sm:
                # input DMAs: x on sync (critical), weights/rows on gpsimd
                xt = ph1.tile([P, NJ, C], FP)
                for j in range(NJ):
                    nc.sync.dma_start(out=xt[:, j, :],
                                      in_=x_in[j * P:(j + 1) * P, :])
                wts = {}
                for nm in ("wk", "wv", "wq"):
                    wts[nm] = ph1.tile([P, NCH, C], BF, tag=f"t_{nm}",
                                       name=f"t_{nm}")
                    nc.scalar.dma_start(
                        out=wts[nm][:],
                        in_=w_in[nm][:].rearrange("(c p) k -> p c k", p=P),
                    )
                rows = {}
                for nm in ("bk", "bv", "bq"):
                    rows[nm] = ph1.tile([1, C], BF, tag=f"r_{nm}", name=f"r_{nm}")
                    nc.scalar.dma_start(out=rows[nm][:], in_=row_in[nm][:])
                cols = {}
                for nm in ("gk", "bek", "gq", "beq"):
                    cols[nm] = ph1.tile([P, NCH], FP, tag=f"c_{nm}",
                                        name=f"c_{nm}")
                    nc.scalar.dma_start(
                        out=cols[nm][:],
                        in_=row_in[nm][0, :].rearrange("(c p) -> p c", p=P),
                    )
                nc.vector.memset(vaug[:, :, :, D:VW], 1.0)

                # s = 2*silu(LN(x)) -> bf16 (0.5 folded into wq/wk/wv)
                st = ph1.tile([P, NJ, C], BF)
                for j in range(NJ):
                    stats = ph1sm.tile([P, 6], FP, tag="xst", name="xst")
                    nc.vector.bn_stats(stats[:], xt[:, j, :])
                    aggr = ph1sm.tile([P, 2], FP, tag="xag", name="xag")
                    nc.vector.bn_aggr(aggr[:], stats[:])
                    rsj = _fast_rsqrt(nc, ph1sm, aggr[:, 1:2], f"x{j}")
                    nmj = ph1sm.tile([P, 1], FP, tag="xnm", name="xnm")
                    nc.vector.scalar_tensor_tensor(
                        nmj[:], aggr[:, 0:1], -1.0, rsj[:], ALU.mult, ALU.mult
                    )
                    zn = ph1sm.tile([P, C], FP, tag="zn", name="zn")
                    nc.vector.tensor_scalar(
                        zn[:], xt[:, j, :], rsj[:], nmj[:], ALU.mult, ALU.add,
                    )
                    th = ph1sm.tile([P, C], FP, tag="th", name="th")
                    nc.scalar.activation(th[:], zn[:], AF.Tanh,
                                         bias=0.0, scale=0.5)
                    nc.vector.scalar_tensor_tensor(
                        st[:, j, :], th[:], 1.0, zn[:], ALU.add, ALU.mult
                    )

                # transpose s -> sT [c partitions, tokens]
                sT = ph1.tile([P, NCH, NJ, P], BF)
                for j in range(NJ):
                    ptr = ph1tr.tile([P, NCH, P], BF, tag="tr", name="tr")
                    for cc in range(NCH):
                        nc.tensor.transpose(
                            ptr[:, cc, :], st[:, j, cc * P:(cc + 1) * P], ident[:]
                        )
                    nc.vector.tensor_copy(sT[:, :, j, :], ptr[:])

                def proj(nm, pool, tag):
                    pq = pool.tile([P, NJ, C], FP, tag=tag, name=tag)
                    for j in range(NJ):
                        for cc in range(NCH):
                            nc.tensor.matmul(
                                pq[:, j, :], sT[:, cc, j, :], wts[nm][:, cc, :],
                                start=(cc == 0), stop=False,
                            )
                        nc.tensor.matmul(
                            pq[:, j, :], ones_r[:], rows[f"b{nm[1]}"][:],
                            start=False, stop=True,
                        )
                    return pq

                # ---- k path ----
                pk = proj("wk", ph1ps, "pkq")
                rsk, nmk = _ln_coeffs(nc, ph1sm, pk[:], "k")
                ynk = ph1.tile([P, NJ, C], BF)
                for j in range(NJ):
                    nc.vector.tensor_scalar(
                        ynk[:, j, :], pk[:, j, :], rsk[:, j:j + 1],
                        nmk[:, j:j + 1], ALU.mult, ALU.add,
                    )

                # ---- v path (per-j psum, overlaps k consumption) ----
                for j in range(NJ):
                    pv = ph1pv.tile([P, C], FP, tag="pv", name="pv")
                    for cc in range(NCH):
                        nc.tensor.matmul(
                            pv[:], sT[:, cc, j, :], wts["wv"][:, cc, :],
                            start=(cc == 0), stop=False,
                        )
                    nc.tensor.matmul(pv[:], ones_r[:], rows["bv"][:],
                                     start=False, stop=True)
                    nc.vector.tensor_copy(
                        vaug[:, j, :, 0:D],
                        pv[:].rearrange("p (h d) -> p h d", h=H),
                    )

                # k transposes + affine, pair-major; bounce + AllGathers per
                # pair so pair 0's attention inputs land first.
                for cc in range(NCH):
                    ptr = ph1tr.tile([P, NJ, P], BF, tag="tr", name="trk")
                    for j in range(NJ):
                        nc.tensor.transpose(
                            ptr[:, j, :], ynk[:, j, cc * P:(cc + 1) * P], ident[:]
                        )
                    nc.vector.tensor_scalar(
                        kT_loc[:, cc], ptr[:], cols["gk"][:, cc:cc + 1],
                        cols["bek"][:, cc:cc + 1], ALU.mult, ALU.add,
                    )
                    nc.sync.dma_start(
                        out=kvb[cc][0:KROWS, :],
                        in_=kT_loc[:, cc].rearrange("p j t -> p (j t)"),
                    )
                    nc.sync.dma_start(
                        out=kvb[cc][KROWS:KVR, :].flatten().rearrange(
                            "(j q w) -> q j w", q=P, w=2 * VW
                        ),
                        in_=vaug[:, :, 2 * cc:2 * cc + 2, :].rearrange(
                            "p j h w -> p j (h w)"
                        ),
                    )
                    nc.gpsimd.collective_compute(
                        "AllGather", ALU.bypass,
                        replica_groups=[list(range(NC))],
                        ins=[kvb[cc][:].opt()],
                        outs=[kvg[cc][:].opt()],
                    )

                # ---- q path (overlaps the collectives); gq/beq pre-scaled ----
                pq = proj("wq", ph1ps, "pkq")
                rsq, nmq = _ln_coeffs(nc, ph1sm, pq[:], "q")
                ynq = ph1.tile([P, NJ, C], BF)
                for j in range(NJ):
                    nc.vector.tensor_scalar(
                        ynq[:, j, :], pq[:, j, :], rsq[:, j:j + 1],
                        nmq[:, j:j + 1], ALU.mult, ALU.add,
                    )
                for cc in range(NCH):
                    ptr = ph1tr.tile([P, NJ, P], BF, tag="tr", name="trq")
                    for j in range(NJ):
                        nc.tensor.transpose(
                            ptr[:, j, :], ynq[:, j, cc * P:(cc + 1) * P], ident[:]
                        )
                    nc.vector.tensor_scalar(
                        qT[cc][:], ptr[:], cols["gq"][:, cc:cc + 1],
                        cols["beq"][:, cc:cc + 1], ALU.mult, ALU.add,
                    )

            # ---------------- phase 2: attention ----------------
            blocks = [list(range(i, min(i + BLK, CHUNKS)))
                      for i in range(0, CHUNKS, BLK)]

            with tc.tile_pool(name="att", bufs=2) as att, \
                 tc.tile_pool(name="attsc", bufs=1, space="PSUM") as attsc, \
                 tc.tile_pool(name="attac", bufs=1, space="PSUM") as attac, \
                 tc.tile_pool(name="attsm", bufs=3) as attsm:
                heat = attsc.tile([P, TLOC], FP, tag="sc0", name="heat")
                for _ in range(200):
                    nc.tensor.matmul(heat[:, 0:P], ident[:], ident[:],
                                     start=True, stop=True)
                for pair in range(H // 2):
                    # k^T rows for this pair, all ranks: [128, 32, 128]
                    ktp = att.tile([P, CHUNKS, P], F8, tag="ktp", name="ktp")
                    nc.sync.dma_start(
                        out=ktp[:].rearrange("p (r j) q -> p r (j q)", r=NC),
                        in_=kvg[pair][:].rearrange(
                            "(r c) t -> c r t", c=KVR)[0:P],
                    )
                    # v_aug for both heads: [128, 32, 2*VW]
                    vap = att.tile([P, CHUNKS, 2 * VW], F8, tag="vap", name="vap")
                    for r in range(NC):
                        nc.sync.dma_start(
                            out=vap[:, 4 * r:4 * r + 4, :],
                            in_=kvg[pair][
                                KVR * r + KROWS:KVR * (r + 1), :
                            ].flatten().rearrange(
                                "(j q w) -> q j w", q=P, w=2 * VW
                            ),
                        )

                    oacc = [
                        attac.tile([VW, TLOC], FP, tag=f"oacc{i}", name=f"oacc{i}")
                        for i in range(2)
                    ]
                    qTp = qT[pair]
                    prev = None  # (blk, pex pair)
                    for blk in blocks:
                        nb = len(blk)
                        psc = [
                            attsc.tile([P, BLK, TLOC], FP, tag=f"sc{i}",
                                       name=f"sc{i}")
                            for i in range(2)
                        ]
                        for i, cc in enumerate(blk):
                            for hh in range(2):
                                o = D * hh
                                nc.tensor.matmul(
                                    psc[hh][:, i, :],
                                    ktp[o:o + D, cc, :],
                                    qTp[o:o + D, :, :],
                                    start=True, stop=True,
                                )
                        pex = [
                            attsm.tile([P, BLK, TLOC], BF, tag=f"pex{i}",
                                       name=f"pex{i}")
                            for i in range(2)
                        ]
                        pxs = []
                        for hh in range(2):
                            if (pair, hh) in EXP_DVE:
                                # DVE exps nb-1 chunks (Schraudolph), ACT the
                                # last one into its OWN tile (two engines
                                # writing one tile serializes on a false WAW).
                                dnb = nb - 1
                                eb = attsm.tile([P, BLK, TLOC], I32,
                                                tag=f"eb{hh}", name=f"eb{hh}")
                                nc.vector.tensor_scalar(
                                    eb[:, 0:dnb, :], psc[hh][:, 0:dnb, :],
                                    SCH_A, SCH_B, ALU.mult, ALU.add,
                                )
                                nc.vector.tensor_copy(
                                    pex[hh][:, 0:dnb, :],
                                    eb[:, 0:dnb, :].bitcast(FP),
                                )
                                pxa = attsm.tile([P, TLOC], BF,
                                                 tag=f"pxa{hh}",
                                                 name=f"pxa{hh}")
                                nc.scalar.activation(
                                    pxa[:], psc[hh][:, dnb:nb, :], AF.Exp,
                                )
                                pxs.append(
                                    [pex[hh][:, i, :] for i in range(dnb)]
                                    + [pxa[:]]
                                )
                            else:
                                nc.scalar.activation(
                                    pex[hh][:, 0:nb, :], psc[hh][:, 0:nb, :],
                                    AF.Exp,
                                )
                                pxs.append(
                                    [pex[hh][:, i, :] for i in range(nb)]
                                )
                        if prev is not None:
                            pblk, ppx = prev
                            for i, cc in enumerate(pblk):
                                for hh in range(2):
                                    nc.tensor.matmul(
                                        oacc[hh][:],
                                        vap[:, cc, hh * VW:(hh + 1) * VW],
                                        ppx[hh][i],
                                        start=(cc == 0), stop=False,
                                    )
                        prev = (blk, pxs)
                    pblk, ppx = prev
                    for i, cc in enumerate(pblk):
                        for hh in range(2):
                            nc.tensor.matmul(
                                oacc[hh][:],
                                vap[:, cc, hh * VW:(hh + 1) * VW],
                                ppx[hh][i],
                                start=False, stop=(cc == CHUNKS - 1),
                            )

                    for hh in range(2):
                        # sums -> bf16 row, replicate to 64 partitions via a
                        # K=1 matmul, 1/x, normalize, 2*silu into soT.
                        smb = attsm.tile([1, TLOC], BF, tag=f"smb{hh}",
                                         name=f"smb{hh}")
                        nc.vector.tensor_copy(smb[:], oacc[hh][D:VW, :])
                        srep = attsc.tile([D, TLOC], FP, tag=f"sc{hh}",
                                          name=f"srep{hh}")
                        nc.tensor.matmul(srep[:], ones_r[:, 0:D], smb[:],
                                         start=True, stop=True)
                        ssb = attsm.tile([D, TLOC], FP, tag=f"ssb{hh}",
                                         name=f"ssb{hh}")
                        nc.scalar.copy(ssb[:], srep[:])
                        rrep = attsm.tile([D, TLOC], FP, tag=f"rr{hh}",
                                          name=f"rr{hh}")
                        nc.vector.reciprocal_approx_fast(rrep[:], ssb[:])
                        onrm = attsm.tile([D, TLOC], FP, tag=f"on{hh}",
                                          name=f"on{hh}")
                        nc.vector.tensor_mul(onrm[:], oacc[hh][0:D, :], rrep[:])
                        o = D * hh
                        th = attsm.tile([D, TLOC], FP, tag=f"sth{hh}",
                                        name=f"sth{hh}")
                        nc.scalar.activation(th[:], onrm[:], AF.Tanh,
                                             bias=0.0, scale=0.5)
                        nc.vector.scalar_tensor_tensor(
                            soT[pair][o:o + D, :, :], th[:], 1.0, onrm[:],
                            ALU.add, ALU.mult,
                        )

            # ---------------- phase 3: output projection ----------------
            with tc.tile_pool(name="ph3ps", bufs=2, space="PSUM") as ph3ps, \
                 tc.tile_pool(name="ph3", bufs=2) as ph3:
                wo_t = ph3.tile([P, NCH, C], BF)
                nc.scalar.dma_start(
                    out=wo_t[:],
                    in_=w_in["wo"][:].rearrange("(c p) k -> p c k", p=P),
                )
                bo_t = ph3.tile([1, C], BF)
                nc.scalar.dma_start(out=bo_t[:], in_=row_in["bo"][:])
                for j in range(NJ):
                    po = ph3ps.tile([P, C], FP, tag="po", name="po")
                    for cc in range(NCH):
                        nc.tensor.matmul(
                            po[:], soT[cc][:, j, :], wo_t[:, cc, :],
                            start=(cc == 0), stop=False,
                        )
                    nc.tensor.matmul(po[:], ones_r[:], bo_t[:],
                                     start=False, stop=True)
                    osb = ph3.tile([P, C], FP, tag="osb", name="osb")
                    nc.vector.tensor_copy(osb[:], po[:])
                    nc.scalar.dma_start(out=out_ext[j * P:(j + 1) * P, :],
                                        in_=osb[:])

    nc.compile()
    return nc


def prepare_in_maps(inputs):
    """Host-side preprocessing: bf16 weight casts (with the silu 0.5 fold),
    query-scale fold into g/be, per-core x shards."""
    import ml_dtypes
    bf16 = ml_dtypes.bfloat16

    x = np.asarray(inputs["x"], dtype=np.float32)
    assert x.shape == (1, N, C)
    scale = np.float32(INNER ** -0.5)

    def wb(a, mul):
        return np.ascontiguousarray(
            (np.asarray(a, np.float32) * mul).astype(bf16)
        )

    def rowb(a):
        return np.ascontiguousarray(
            np.asarray(a, np.float32).reshape(1, C).astype(bf16)
        )

    def rowf(a):
        return np.ascontiguousarray(np.asarray(a, np.float32).reshape(1, C))

    common = {
        # 0.5 folds: s and silu(o) are computed as 2*silu(.)
        "wq": wb(inputs["w_q"], 0.5),
        "wk": wb(inputs["w_k"], 0.5),
        "wv": wb(inputs["w_v"], 0.5),
        "wo": wb(inputs["w_o"], 0.5),
        "bq": rowb(inputs["b_q"]),
        "bk": rowb(inputs["b_k"]),
        "bv": rowb(inputs["b_v"]),
        "bo": rowb(inputs["b_o"]),
        "gq": rowf(np.asarray(inputs["g_q"], np.float32) * scale),
        "beq": rowf(np.asarray(inputs["be_q"], np.float32) * scale),
        "gk": rowf(inputs["g_k"]),
        "bek": rowf(inputs["be_k"]),
    }
    in_maps = []
    for r in range(NC):
        m = dict(common)
        m["x"] = np.ascontiguousarray(x[0, r * TLOC:(r + 1) * TLOC, :])
        in_maps.append(m)
    return in_maps


def kernel(**inputs):
    x = np.asarray(inputs["x"], dtype=np.float32)
    B = x.shape[0]
    if "nc" not in _CACHE:
        _CACHE["nc"] = build_graph()
    nc = _CACHE["nc"]
    in_maps = prepare_in_maps(inputs)
    res = run_bass_kernel_spmd(nc, in_maps, core_ids=list(range(NC)))
    out = np.concatenate([res.results[r]["out"] for r in range(NC)], axis=0)
    return out.reshape(B, N, C)


if __name__ == "__main__":
    sys.path.insert(0, "/root/problem")
    import reference

    inputs = {k: np.asarray(v) for k, v in reference.setup_inputs().items()}
    expected = np.asarray(reference.reference(**reference.setup_inputs()))
    actual = kernel(**inputs)
    err = np.linalg.norm(actual - expected) / np.linalg.norm(expected)
    print("Relative error:", err)


# revision 39
# speedup vs baseline: 1.0914x; 1.0914x over previous
"""Distributed Trainium2 kernel for the LN->silu->QKV(+LN on q,k)->attention->silu->proj block.

Sharding: sequence-parallel over 8 NeuronCores. Each core owns 512 of the 4096
tokens; both LayerNorms are per-token so they stay local. Collectives are four
per-head-pair AllGathers of (K^T, V_aug) so attention on pair p can start as
soon as its own gather lands.

Device layout conventions (per core):
  natural    = [token partitions, feature free]
  transposed = [feature partitions, token free]
Scores are computed transposed (S^T = [key, query]) so the softmax denominators
come free from the P@V matmul: V is augmented with a ones column, so the AV
accumulator row 64 is sum_k P. exp() needs no max subtraction: q,k are
LayerNorm outputs and q is scaled by inner^-0.5, so |scores| <~ 3.

Engine split: all matmuls bf16 on PE; LayerNorm rsqrt via a DVE bit-trick
(fast inverse sqrt + 1 Newton step) so ACT only ever needs the exp_and_others
table set (exp + tanh); softmax exp runs on ACT for 5 heads and on DVE
(Schraudolph bit-trick exp, ~3% max err) for 3 heads to balance the two
engines. silu is z*(1+tanh(z/2)) = 2*silu(z) with the 0.5 folded into the
host-scaled weights.

Attention inner loop is software-pipelined (scores(b); exp(b); AV(b-1)) so the
in-order PE queue never stalls waiting for exp.
"""

import sys
import numpy as np

sys.path.insert(0, "/opt/trn_rl_repo")

import concourse.bacc as bacc  # noqa: E402
import concourse.tile as tile  # noqa: E402
from concourse import mybir, masks  # noqa: E402
from concourse.bass_utils import run_bass_kernel_spmd  # noqa: E402

FP = mybir.dt.float32
BF = mybir.dt.bfloat16
F8 = mybir.dt.float8e4
I32 = mybir.dt.int32
AF = mybir.ActivationFunctionType
ALU = mybir.AluOpType

NC = 8          # cores
P = 128         # partitions
N = 4096        # sequence
C = 512         # channels
INNER = 512     # heads * dim_head
H = 8           # heads
D = 64          # dim per head
TLOC = N // NC  # tokens per core (512)
NJ = TLOC // P  # token tiles per core (4)
NCH = C // P    # channel chunks (4)
VW = D + 1      # augmented v width (65)
BLK = 3         # key chunks per score block (3 PSUM banks per head)
CHUNKS = N // P  # 32 key chunks

# (pair, hh) whose softmax exp runs on DVE via the Schraudolph bit trick.
EXP_DVE = {(0, 1), (1, 1), (2, 1), (3, 1)}
SCH_A = float(1 << 23) / float(np.log(2.0))
SCH_B = float(127 * (1 << 23) - 366393)

_CACHE = {}


def _fast_rsqrt(nc, pool, var_ap, tag):
    """rs = 1/sqrt(var) on DVE: magic-constant seed + one Newton step.
    var_ap: [P, NJ] fp32 SBUF. Returns [P, NJ] fp32 tile. Max rel err ~0.2%."""
    shp = list(var_ap.shape)
    ish = pool.tile(shp, I32, tag=f"{tag}_i", name=f"{tag}_i")
    nc.vector.tensor_scalar(
        ish[:], var_ap.bitcast(I32), 1, None, ALU.logical_shift_right
    )
    r0 = pool.tile(shp, I32, tag=f"{tag}_r0", name=f"{tag}_r0")
    nc.vector.tensor_scalar(r0[:], ish[:], -1, 0x5F3759DF, ALU.mult, ALU.add)
    vr = pool.tile(shp, FP, tag=f"{tag}_vr", name=f"{tag}_vr")
    nc.vector.tensor_tensor(vr[:], var_ap, r0[:].bitcast(FP), ALU.mult)
    vrr = pool.tile(shp, FP, tag=f"{tag}_v2", name=f"{tag}_v2")
    nc.vector.tensor_tensor(vrr[:], vr[:], r0[:].bitcast(FP), ALU.mult)
    h = pool.tile(shp, FP, tag=f"{tag}_h", name=f"{tag}_h")
    nc.vector.tensor_scalar(h[:], vrr[:], -0.5, 1.5, ALU.mult, ALU.add)
    rs = pool.tile(shp, FP, tag=f"{tag}_rs", name=f"{tag}_rs")
    nc.vector.tensor_tensor(rs[:], h[:], r0[:].bitcast(FP), ALU.mult)
    return rs


def _ln_coeffs(nc, pool, src_ap, tag):
    """Per-token LN coefficients for src_ap [128, NJ, 512]: returns
    (rs, nmr) [128, NJ]: rsqrt(var) and -mean*rsqrt (eps skipped; var >> eps)."""
    stats = pool.tile([P, NJ, 6], FP, tag=f"{tag}_st", name=f"{tag}_st")
    for j in range(NJ):
        nc.vector.bn_stats(stats[:, j, :], src_ap[:, j, :])
    aggr = pool.tile([P, NJ, 2], FP, tag=f"{tag}_ag", name=f"{tag}_ag")
    for j in range(NJ):
        nc.vector.bn_aggr(aggr[:, j, :], stats[:, j, :])
    rs = _fast_rsqrt(nc, pool, aggr[:, :, 1], f"{tag}_q")
    nmr = pool.tile([P, NJ], FP, tag=f"{tag}_nm", name=f"{tag}_nm")
    nc.vector.scalar_tensor_tensor(
        nmr[:], aggr[:, :, 0], -1.0, rs[:], ALU.mult, ALU.mult
    )
    return rs, nmr


def build_graph():
    nc = bacc.Bacc("TRN2", target_bir_lowering=False, debug=False, num_devices=NC)

    x_in = nc.dram_tensor("x", [TLOC, C], FP, kind="ExternalInput")
    w_in = {}
    for nm in ("wq", "wk", "wv", "wo"):
        w_in[nm] = nc.dram_tensor(nm, [C, C], BF, kind="ExternalInput")
    row_in = {}
    for nm in ("bq", "bk", "bv", "bo"):
        row_in[nm] = nc.dram_tensor(nm, [1, C], BF, kind="ExternalInput")
    for nm in ("gq", "beq", "gk", "bek"):
        row_in[nm] = nc.dram_tensor(nm, [1, C], FP, kind="ExternalInput")
    out_ext = nc.dram_tensor("out", [TLOC, C], FP, kind="ExternalOutput")

    # per-pair collective bounce/gather buffers. fp8 halves the wire bytes
    # (the AllGather is the latency+bandwidth wall) and its 1-byte elements
    # let k^T [128,512] and v_aug [512,130] pack evenly into one 512B-wide
    # buffer, so each pair needs only ONE collective (control overhead is
    # ~8us per collective).
    KROWS = P            # k^T block rows in kvb
    VROWS = (TLOC * 2 * VW) // C  # 130 rows of flattened v bytes
    KVR = KROWS + VROWS  # 258
    kvb = [nc.dram_tensor(f"kvb{p}", [KVR, C], F8) for p in range(H // 2)]
    kvg = [nc.dram_tensor(f"kvg{p}", [NC * KVR, C], F8, addr_space="Shared")
           for p in range(H // 2)]

    with tile.TileContext(nc) as tc:
        with tc.tile_pool(name="persist", bufs=1) as pers:
            ident = pers.tile([P, P], BF)
            masks.make_identity(nc, ident[:])
            ones_r = pers.tile([1, P], BF)
            nc.vector.memset(ones_r[:], 1.0)

            # qT[c]: [inner partitions, token free]; chunk c = head pair c
            qT = [pers.tile([P, NJ, P], BF, tag=f"qT{c}", name=f"qT{c}")
                  for c in range(NCH)]
            soT = [pers.tile([P, NJ, P], BF, tag=f"soT{c}", name=f"soT{c}")
                   for c in range(NCH)]
            kT_loc = pers.tile([P, NCH, NJ, P], F8)   # k^T local by chunk
            vaug = pers.tile([P, NJ, H, VW], F8)      # v augmented, natural

            # ---------------- phase 1: local projections ----------------
            with tc.tile_pool(name="ph1", bufs=1) as ph1, \
                 tc.tile_pool(name="ph1ps", bufs=1, space="PSUM") as ph1ps, \
                 tc.tile_pool(name="ph1pv", bufs=2, space="PSUM") as ph1pv, \
                 tc.tile_pool(name="ph1tr", bufs=2, space="PSUM") as ph1tr, \
                 tc.tile_pool(name="ph1sm", bufs=2) as ph1sm:
                # input DMAs: x on sync (critical), weights/rows on gpsimd
                xt = ph1.tile([P, NJ, C], FP)
                for j in range(NJ):
                    nc.sync.dma_start(out=xt[:, j, :],
                                      in_=x_in[j * P:(j + 1) * P, :])
                wts = {}
                for nm in ("wk", "wv", "wq"):
                    wts[nm] = ph1.tile([P, NCH, C], BF, tag=f"t_{nm}",
                                       name=f"t_{nm}")
                    nc.scalar.dma_start(
                        out=wts[nm][:],
                        in_=w_in[nm][:].rearrange("(c p) k -> p c k", p=P),
                    )
                rows = {}
                for nm in ("bk", "bv", "bq"):
                    rows[nm] = ph1.tile([1, C], BF, tag=f"r_{nm}", name=f"r_{nm}")
                    nc.scalar.dma_start(out=rows[nm][:], in_=row_in[nm][:])
                cols = {}
                for nm in ("gk", "bek", "gq", "beq"):
                    cols[nm] = ph1.tile([P, NCH], FP, tag=f"c_{nm}",
                                        name=f"c_{nm}")
                    nc.scalar.dma_start(
                        out=cols[nm][:],
                        in_=row_in[nm][0, :].rearrange("(c p) -> p c", p=P),
                    )
                nc.vector.memset(vaug[:, :, :, D:VW], 1.0)

                # s = 2*silu(LN(x)) -> bf16 (0.5 folded into wq/wk/wv)
                st = ph1.tile([P, NJ, C], BF)
                for j in range(NJ):
                    stats = ph1sm.tile([P, 6], FP, tag="xst", name="xst")
                    nc.vector.bn_stats(stats[:], xt[:, j, :])
                    aggr = ph1sm.tile([P, 2], FP, tag="xag", name="xag")
                    nc.vector.bn_aggr(aggr[:], stats[:])
                    rsj = _fast_rsqrt(nc, ph1sm, aggr[:, 1:2], f"x{j}")
                    nmj = ph1sm.tile([P, 1], FP, tag="xnm", name="xnm")
                    nc.vector.scalar_tensor_tensor(
                        nmj[:], aggr[:, 0:1], -1.0, rsj[:], ALU.mult, ALU.mult
                    )
                    zn = ph1sm.tile([P, C], FP, tag="zn", name="zn")
                    nc.vector.tensor_scalar(
                        zn[:], xt[:, j, :], rsj[:], nmj[:], ALU.mult, ALU.add,
                    )
                    th = ph1sm.tile([P, C], FP, tag="th", name="th")
                    nc.scalar.activation(th[:], zn[:], AF.Tanh,
                                         bias=0.0, scale=0.5)
                    nc.vector.scalar_tensor_tensor(
                        st[:, j, :], th[:], 1.0, zn[:], ALU.add, ALU.mult
                    )

                # transpose s -> sT [c partitions, tokens]
                sT = ph1.tile([P, NCH, NJ, P], BF)
                for j in range(NJ):
                    ptr = ph1tr.tile([P, NCH, P], BF, tag="tr", name="tr")
                    for cc in range(NCH):
                        nc.tensor.transpose(
                            ptr[:, cc, :], st[:, j, cc * P:(cc + 1) * P], ident[:]
                        )
                    nc.vector.tensor_copy(sT[:, :, j, :], ptr[:])

                def proj(nm, pool, tag):
                    pq = pool.tile([P, NJ, C], FP, tag=tag, name=tag)
                    for j in range(NJ):
                        for cc in range(NCH):
                            nc.tensor.matmul(
                                pq[:, j, :], sT[:, cc, j, :], wts[nm][:, cc, :],
                                start=(cc == 0), stop=False,
                            )
                        nc.tensor.matmul(
                            pq[:, j, :], ones_r[:], rows[f"b{nm[1]}"][:],
                            start=False, stop=True,
                        )
                    return pq

                # ---- k path ----
                pk = proj("wk", ph1ps, "pkq")
                rsk, nmk = _ln_coeffs(nc, ph1sm, pk[:], "k")
                ynk = ph1.tile([P, NJ, C], BF)
                for j in range(NJ):
                    nc.vector.tensor_scalar(
                        ynk[:, j, :], pk[:, j, :], rsk[:, j:j + 1],
                        nmk[:, j:j + 1], ALU.mult, ALU.add,
                    )

                # ---- v path (per-j psum, overlaps k consumption) ----
                for j in range(NJ):
                    pv = ph1pv.tile([P, C], FP, tag="pv", name="pv")
                    for cc in range(NCH):
                        nc.tensor.matmul(
                            pv[:], sT[:, cc, j, :], wts["wv"][:, cc, :],
                            start=(cc == 0), stop=False,
                        )
                    nc.tensor.matmul(pv[:], ones_r[:], rows["bv"][:],
                                     start=False, stop=True)
                    nc.vector.tensor_copy(
                        vaug[:, j, :, 0:D],
                        pv[:].rearrange("p (h d) -> p h d", h=H),
                    )

                # k transposes + affine, pair-major; bounce + AllGathers per
                # pair so pair 0's attention inputs land first.
                for cc in range(NCH):
                    ptr = ph1tr.tile([P, NJ, P], BF, tag="tr", name="trk")
                    for j in range(NJ):
                        nc.tensor.transpose(
                            ptr[:, j, :], ynk[:, j, cc * P:(cc + 1) * P], ident[:]
                        )
                    nc.vector.tensor_scalar(
                        kT_loc[:, cc], ptr[:], cols["gk"][:, cc:cc + 1],
                        cols["bek"][:, cc:cc + 1], ALU.mult, ALU.add,
                    )
                    nc.sync.dma_start(
                        out=kvb[cc][0:KROWS, :],
                        in_=kT_loc[:, cc].rearrange("p j t -> p (j t)"),
                    )
                    nc.sync.dma_start(
                        out=kvb[cc][KROWS:KVR, :].flatten().rearrange(
                            "(j q w) -> q j w", q=P, w=2 * VW
                        ),
                        in_=vaug[:, :, 2 * cc:2 * cc + 2, :].rearrange(
                            "p j h w -> p j (h w)"
                        ),
                    )
                    nc.gpsimd.collective_compute(
                        "AllGather", ALU.bypass,
                        replica_groups=[list(range(NC))],
                        ins=[kvb[cc][:].opt()],
                        outs=[kvg[cc][:].opt()],
                    )

                # ---- q path (overlaps the collectives); gq/beq pre-scaled ----
                pq = proj("wq", ph1ps, "pkq")
                rsq, nmq = _ln_coeffs(nc, ph1sm, pq[:], "q")
                ynq = ph1.tile([P, NJ, C], BF)
                for j in range(NJ):
                    nc.vector.tensor_scalar(
                        ynq[:, j, :], pq[:, j, :], rsq[:, j:j + 1],
                        nmq[:, j:j + 1], ALU.mult, ALU.add,
                    )
                for cc in range(NCH):
                    ptr = ph1tr.tile([P, NJ, P], BF, tag="tr", name="trq")
                    for j in range(NJ):
                        nc.tensor.transpose(
                            ptr[:, j, :], ynq[:, j, cc * P:(cc + 1) * P], ident[:]
                        )
                    nc.vector.tensor_scalar(
                        qT[cc][:], ptr[:], cols["gq"][:, cc:cc + 1],
                        cols["beq"][:, cc:cc + 1], ALU.mult, ALU.add,
                    )

            # ---------------- phase 2: attention ----------------
            blocks = [list(range(i, min(i + BLK, CHUNKS)))
                      for i in range(0, CHUNKS, BLK)]

            with tc.tile_pool(name="att", bufs=2) as att, \
                 tc.tile_pool(name="attsc", bufs=1, space="PSUM") as attsc, \
                 tc.tile_pool(name="attac", bufs=1, space="PSUM") as attac, \
                 tc.tile_pool(name="attsm", bufs=3) as attsm:
                heat = attsc.tile([P, TLOC], FP, tag="sc0", name="heat")
                for _ in range(200):
                    nc.tensor.matmul(heat[:, 0:P], ident[:], ident[:],
                                     start=True, stop=True)
                for pair in range(H // 2):
                    # k^T rows for this pair, all ranks: [128, 32, 128]
                    ktp = att.tile([P, CHUNKS, P], F8, tag="ktp", name="ktp")
                    nc.sync.dma_start(
                        out=ktp[:].rearrange("p (r j) q -> p r (j q)", r=NC),
                        in_=kvg[pair][:].rearrange(
                            "(r c) t -> c r t", c=KVR)[0:P],
                    )
                    # v_aug for both heads: [128, 32, 2*VW]
                    vap = att.tile([P, CHUNKS, 2 * VW], F8, tag="vap", name="vap")
                    for r in range(NC):
                        nc.sync.dma_start(
                            out=vap[:, 4 * r:4 * r + 4, :],
                            in_=kvg[pair][
                                KVR * r + KROWS:KVR * (r + 1), :
                            ].flatten().rearrange(
                                "(j q w) -> q j w", q=P, w=2 * VW
                            ),
                        )

                    oacc = [
                        attac.tile([VW, TLOC], FP, tag=f"oacc{i}", name=f"oacc{i}")
                        for i in range(2)
                    ]
                    qTp = qT[pair]
                    prev = None  # (blk, pex pair)
                    for blk in blocks:
                        nb = len(blk)
                        psc = [
                            attsc.tile([P, BLK, TLOC], FP, tag=f"sc{i}",
                                       name=f"sc{i}")
                            for i in range(2)
                        ]
                        for i, cc in enumerate(blk):
                            for hh in range(2):
                                o = D * hh
                                nc.tensor.matmul(
                                    psc[hh][:, i, :],
                                    ktp[o:o + D, cc, :],
                                    qTp[o:o + D, :, :],
                                    start=True, stop=True,
                                )
                        pex = [
                            attsm.tile([P, BLK, TLOC], BF, tag=f"pex{i}",
                                       name=f"pex{i}")
                            for i in range(2)
                        ]
                        for hh in range(2):
                            if (pair, hh) in EXP_DVE:
                                eb = attsm.tile([P, BLK, TLOC], I32,
                                                tag=f"eb{hh}", name=f"eb{hh}")
                                nc.vector.tensor_scalar(
                                    eb[:, 0:nb, :], psc[hh][:, 0:nb, :],
                                    SCH_A, SCH_B, ALU.mult, ALU.add,
                                )
                                nc.vector.tensor_copy(
                                    pex[hh][:, 0:nb, :],
                                    eb[:, 0:nb, :].bitcast(FP),
                                )
                            else:
                                nc.scalar.activation(
                                    pex[hh][:, 0:nb, :], psc[hh][:, 0:nb, :],
                                    AF.Exp,
                                )
                        if prev is not None:
                            pblk, ppex = prev
                            for i, cc in enumerate(pblk):
                                for hh in range(2):
                                    nc.tensor.matmul(
                                        oacc[hh][:],
                                        vap[:, cc, hh * VW:(hh + 1) * VW],
                                        ppex[hh][:, i, :],
                                        start=(cc == 0), stop=False,
                                    )
                        prev = (blk, pex)
                    pblk, ppex = prev
                    for i, cc in enumerate(pblk):
                        for hh in range(2):
                            nc.tensor.matmul(
                                oacc[hh][:],
                                vap[:, cc, hh * VW:(hh + 1) * VW],
                                ppex[hh][:, i, :],
                                start=False, stop=(cc == CHUNKS - 1),
                            )

                    for hh in range(2):
                        # sums -> bf16 row, replicate to 64 partitions via a
                        # K=1 matmul, 1/x, normalize, 2*silu into soT.
                        smb = attsm.tile([1, TLOC], BF, tag=f"smb{hh}",
                                         name=f"smb{hh}")
                        nc.vector.tensor_copy(smb[:], oacc[hh][D:VW, :])
                        srep = attsc.tile([D, TLOC], FP, tag=f"sc{hh}",
                                          name=f"srep{hh}")
                        nc.tensor.matmul(srep[:], ones_r[:, 0:D], smb[:],
                                         start=True, stop=True)
                        ssb = attsm.tile([D, TLOC], FP, tag=f"ssb{hh}",
                                         name=f"ssb{hh}")
                        nc.scalar.copy(ssb[:], srep[:])
                        rrep = attsm.tile([D, TLOC], FP, tag=f"rr{hh}",
                                          name=f"rr{hh}")
                        nc.vector.reciprocal_approx_fast(rrep[:], ssb[:])
                        onrm = attsm.tile([D, TLOC], FP, tag=f"on{hh}",
                                          name=f"on{hh}")
                        nc.vector.tensor_mul(onrm[:], oacc[hh][0:D, :], rrep[:])
                        o = D * hh
                        th = attsm.tile([D, TLOC], FP, tag=f"sth{hh}",
                                        name=f"sth{hh}")
                        nc.scalar.activation(th[:], onrm[:], AF.Tanh,
                                             bias=0.0, scale=0.5)
                        nc.vector.scalar_tensor_tensor(
                            soT[pair][o:o + D, :, :], th[:], 1.0, onrm[:],
                            ALU.add, ALU.mult,
                        )

            # ---------------- phase 3: output projection ----------------
            with tc.tile_pool(name="ph3ps", bufs=2, space="PSUM") as ph3ps, \
                 tc.tile_pool(name="ph3", bufs=2) as ph3:
                wo_t = ph3.tile([P, NCH, C], BF)
                nc.scalar.dma_start(
                    out=wo_t[:],
                    in_=w_in["wo"][:].rearrange("(c p) k -> p c k", p=P),
                )
                bo_t = ph3.tile([1, C], BF)
                nc.scalar.dma_start(out=bo_t[:], in_=row_in["bo"][:])
                for j in range(NJ):
                    po = ph3ps.tile([P, C], FP, tag="po", name="po")
                    for cc in range(NCH):
                        nc.tensor.matmul(
                            po[:], soT[cc][:, j, :], wo_t[:, cc, :],
                            start=(cc == 0), stop=False,
                        )
                    nc.tensor.matmul(po[:], ones_r[:], bo_t[:],
                                     start=False, stop=True)
                    osb = ph3.tile([P, C], FP, tag="osb", name="osb")
                    nc.vector.tensor_copy(osb[:], po[:])
                    nc.scalar.dma_start(out=out_ext[j * P:(j + 1) * P, :],
                                        in_=osb[:])

    nc.compile()
    return nc


def prepare_in_maps(inputs):
    """Host-side preprocessing: bf16 weight casts (with the silu 0.5 fold),
    query-scale fold into g/be, per-core x shards."""
    import ml_dtypes
    bf16 = ml_dtypes.bfloat16

    x = np.asarray(inputs["x"], dtype=np.float32)
    assert x.shape == (1, N, C)
    scale = np.float32(INNER ** -0.5)

    def wb(a, mul):
        return np.ascontiguousarray(
            (np.asarray(a, np.float32) * mul).astype(bf16)
        )

    def rowb(a):
        return np.ascontiguousarray(
            np.asarray(a, np.float32).reshape(1, C).astype(bf16)
        )

    def rowf(a):
        return np.ascontiguousarray(np.asarray(a, np.float32).reshape(1, C))

    common = {
        # 0.5 folds: s and silu(o) are computed as 2*silu(.)
        "wq": wb(inputs["w_q"], 0.5),
        "wk": wb(inputs["w_k"], 0.5),
        "wv": wb(inputs["w_v"], 0.5),
        "wo": wb(inputs["w_o"], 0.5),
        "bq": rowb(inputs["b_q"]),
        "bk": rowb(inputs["b_k"]),
        "bv": rowb(inputs["b_v"]),
        "bo": rowb(inputs["b_o"]),
        "gq": rowf(np.asarray(inputs["g_q"], np.float32) * scale),
        "beq": rowf(np.asarray(inputs["be_q"], np.float32) * scale),
        "gk": rowf(inputs["g_k"]),
        "bek": rowf(inputs["be_k"]),
    }
    in_maps = []
    for r in range(NC):
        m = dict(common)
        m["x"] = np.ascontiguousarray(x[0, r * TLOC:(r + 1) * TLOC, :])
        in_maps.append(m)
    return in_maps


def kernel(**inputs):
    x = np.asarray(inputs["x"], dtype=np.float32)
    B = x.shape[0]
    if "nc" not in _CACHE:
        _CACHE["nc"] = build_graph()
    nc = _CACHE["nc"]
    in_maps = prepare_in_maps(inputs)
    res = run_bass_kernel_spmd(nc, in_maps, core_ids=list(range(NC)))
    out = np.concatenate([res.results[r]["out"] for r in range(NC)], axis=0)
    return out.reshape(B, N, C)


if __name__ == "__main__":
    sys.path.insert(0, "/root/problem")
    import reference

    inputs = {k: np.asarray(v) for k, v in reference.setup_inputs().items()}
    expected = np.asarray(reference.reference(**reference.setup_inputs()))
    actual = kernel(**inputs)
    err = np.linalg.norm(actual - expected) / np.linalg.norm(expected)
    print("Relative error:", err)
